# revision 1
# baseline (speedup 1.0000x reference)
"""CMaxPool4d (complex modulus max-pool, K=2 stride 2 over 4 spatial dims) on 8 Trainium2 cores.

Input  : [8, 2, 32, 16, 16, 16, 16] f32  (dim1 = real/imag)
Output : [8, 2, 32, 8, 8, 8, 8] f32      (value of r/i at the max-|z| position of each 2^4 window)

Strategy: data-parallel over batch (core b <- batch b). Per core, a 4-level
pairwise tournament over the 16 window candidates, LSB-first (d4, d3, d2, d1)
so ties resolve to the first (lowest) window index, matching jnp.argmax.
Each merge: mask = (m_hi > m_lo) on DVE; payload (r,i) moved in place with
copy_predicated; m updated with max. m = r^2 + i^2 (ACT Square + add);
sqrt is monotone so it is not needed for the comparison.

The host pre-permute splits the d4/d3 parities into separate contiguous
blocks (slab-local f = d*512 + s*256 + q*64 + o3*8 + o4), so every engine op
reads/writes contiguous runs and GPSIMD (flat-1D-AP-only) can take the adds.
Work is organized per chunk (8 channels) and within a chunk per half
(a=0 / a=1), which are fully independent through the D2 level, so engines
pipeline across halves and chunks. Winners land at the front of slabs
j=0/1 and are DMAed out directly.
"""

import os
import sys

import numpy as np

for p in ("/opt/trn_rl_repo", "/opt/pypackages", "/root/.axon_site", "/root/.axon_site/_ro/trn_rl_repo", "/root/.axon_site/_ro/pypackages"):
    if os.path.isdir(p) and p not in sys.path:
        sys.path.append(p)

from concourse import bacc, mybir  # noqa: E402
from concourse.tile import TileContext  # noqa: E402
from concourse.bass_utils import run_bass_kernel_spmd  # noqa: E402

N_CORES = 8
RI = 2
C = 32
D = 16
O = D // 2
NCH = 8                    # channels per chunk
NCHUNK = int(os.environ.get("K_NCHUNK", C // NCH))
SLAB = 1024                # free elems per slab per partition
XF = 8 * SLAB              # 8192

F32 = mybir.dt.float32
U8 = mybir.dt.uint8

ADD_ENGINE = os.environ.get("K_ADD_ENGINE", "vector")  # vector | gpsimd | mixed
WIDE = os.environ.get("K_WIDE", "0") == "1"               # full-width (4-block) level ops
PRED3D = os.environ.get("K_PRED3D", "1") == "1"   # fused (t, ri, f) preds
SPLIT_DMA = os.environ.get("K_SPLIT_DMA", "1") == "1"
BUFS = int(os.environ.get("K_BUFS", "2"))
BUFS_X = int(os.environ.get("K_BUFS_X", str(BUFS)))
BUFS_MASK = int(os.environ.get("K_BUFS_MASK", str(BUFS)))
OUT_STAGE = os.environ.get("K_OUT_STAGE", "0") == "1"
LOOPS = int(os.environ.get("K_LOOPS", "1"))       # whole-kernel idempotent repeats (bench)
REP_DVE = int(os.environ.get("K_REP_DVE", "1"))
REP_ACT = int(os.environ.get("K_REP_ACT", "1"))
REP_GP = int(os.environ.get("K_REP_GP", "1"))
REP_DMA = int(os.environ.get("K_REP_DMA", "1"))

_COMPILED = None


def _build():
    nc = bacc.Bacc("TRN2", num_devices=N_CORES)
    x_dram = nc.declare_dram_parameter("x", [NCHUNK, 128, XF], F32, isOutput=False)
    y_dram = nc.declare_dram_parameter("y", [NCHUNK, 128, 512], F32, isOutput=True)

    from contextlib import ExitStack
    with TileContext(nc) as tc, ExitStack() as stack:
        pool = stack.enter_context(tc.tile_pool(name="sbuf", bufs=BUFS))
        pool_x = stack.enter_context(tc.tile_pool(name="xpool", bufs=BUFS_X))
        pool_mask = stack.enter_context(tc.tile_pool(name="maskpool", bufs=BUFS_MASK))
        for k in [kk for _ in range(LOOPS) for kk in range(NCHUNK)]:
            X = pool_x.tile([128, XF], F32, tag="X")
            for _ in range(REP_DMA):
                if SPLIT_DMA:
                    nc.sync.dma_start(out=X[:, 0:4096], in_=x_dram[k][:, 0:4096])
                    nc.sync.dma_start(out=X[:, 4096:8192], in_=x_dram[k][:, 4096:8192])
                else:
                    nc.sync.dma_start(out=X[:, :], in_=x_dram[k])

            xtr = X.rearrange("p (t ri f) -> p t ri f", t=4, ri=2)
            M = pool.tile([128, 4096], F32, tag="M")  # 4 t-blocks of 1024
            mt = M.rearrange("p (t f) -> p t f", t=4)

            def pred(t0, n_t, mask_ap, half):
                """X[t0..t0+n_t-1][0:half] <- X[..][half:2*half] where mask."""
                if PRED3D and n_t > 1:
                    mk = mask_ap.rearrange("p (t f) -> p t f", t=n_t)
                    mk = mk.unsqueeze(2).broadcast_to((128, n_t, 2, half))
                    for _ in range(REP_DVE):
                        nc.vector.copy_predicated(
                            xtr[:, t0:t0 + n_t, :, 0:half], mk,
                            xtr[:, t0:t0 + n_t, :, half:2 * half],
                        )
                else:
                    mk2 = mask_ap.rearrange("p (t f) -> p t f", t=n_t)
                    for t in range(n_t):
                        mk = mk2[:, t].unsqueeze(1).broadcast_to((128, 2, half))
                        for _ in range(REP_DVE):
                            nc.vector.copy_predicated(
                                xtr[:, t0 + t, :, 0:half], mk,
                                xtr[:, t0 + t, :, half:2 * half],
                            )

            def level43(t0, n_t, half, mask_pool_shape, tag):
                """one d4/d3-style level on blocks t0..t0+n_t-1: [0:half) vs [half:2half)."""
                maskT = pool_mask.tile([128, n_t * half], U8, tag=tag)
                for _ in range(REP_DVE):
                    nc.vector.tensor_tensor(
                        maskT.rearrange("p (t f) -> p t f", t=n_t),
                        mt[:, t0:t0 + n_t, half:2 * half], mt[:, t0:t0 + n_t, 0:half],
                        mybir.AluOpType.is_gt,
                    )
                pred(t0, n_t, maskT, half)
                for _ in range(REP_DVE):
                    nc.vector.tensor_tensor(
                        mt[:, t0:t0 + n_t, 0:half], mt[:, t0:t0 + n_t, 0:half],
                        mt[:, t0:t0 + n_t, half:2 * half], mybir.AluOpType.max,
                    )

            def level2(t0):
                """b-merge: block t0+1 into t0 on [0:256)."""
                mask2 = pool_mask.tile([128, 256], U8, tag="mask2")
                for _ in range(REP_DVE):
                    nc.vector.tensor_tensor(
                        mask2[:, :], mt[:, t0 + 1, 0:256], mt[:, t0, 0:256],
                        mybir.AluOpType.is_gt,
                    )
                mk = mask2.unsqueeze(1).broadcast_to((128, 2, 256))
                for _ in range(REP_DVE):
                    nc.vector.copy_predicated(
                        xtr[:, t0, :, 0:256], mk, xtr[:, t0 + 1, :, 0:256]
                    )
                for _ in range(REP_DVE):
                    nc.vector.tensor_tensor(
                        mt[:, t0, 0:256], mt[:, t0, 0:256], mt[:, t0 + 1, 0:256],
                        mybir.AluOpType.max,
                    )

            def level2_wide():
                """b-merge full-width: blocks {1,3} into {0,2} on [0:256)."""
                ma = M.rearrange("p (a b f) -> p a b f", a=2, b=2)
                mask2 = pool_mask.tile([128, 512], U8, tag="mask2")
                for _ in range(REP_DVE):
                    nc.vector.tensor_tensor(
                        mask2.rearrange("p (a f) -> p a f", a=2),
                        ma[:, :, 1, 0:256], ma[:, :, 0, 0:256], mybir.AluOpType.is_gt,
                    )
                xa = X.rearrange("p (a b ri f) -> p a b ri f", a=2, b=2, ri=2)
                mk = mask2.rearrange("p (a f) -> p a f", a=2)
                mk = mk.unsqueeze(2).broadcast_to((128, 2, 2, 256))
                for _ in range(REP_DVE):
                    nc.vector.copy_predicated(
                        xa[:, :, 0, :, 0:256], mk, xa[:, :, 1, :, 0:256]
                    )
                for _ in range(REP_DVE):
                    nc.vector.tensor_tensor(
                        ma[:, :, 0, 0:256], ma[:, :, 0, 0:256], ma[:, :, 1, 0:256],
                        mybir.AluOpType.max,
                    )

            for h in range(2):
                # squares (ri-major) and adds
                SQ = pool.tile([128, 4096], F32, tag="SQ")
                for _ in range(REP_ACT):
                    nc.scalar.activation(
                        SQ.rearrange("p (ri b f) -> p b ri f", ri=2, b=2),
                        X[:, h * 4096:(h + 1) * 4096],
                        mybir.ActivationFunctionType.Square,
                    )
                if ADD_ENGINE == "mixed":
                    add_eng = "gpsimd" if h == 0 else "vector"
                else:
                    add_eng = ADD_ENGINE
                for _ in range(REP_GP if add_eng == "gpsimd" else REP_DVE):
                    getattr(nc, add_eng).tensor_tensor(
                        M[:, h * 2048:(h + 1) * 2048], SQ[:, 0:2048], SQ[:, 2048:4096],
                        mybir.AluOpType.add,
                    )
                if not WIDE:
                    t0 = 2 * h
                    level43(t0, 2, 512, None, "mask4")
                    level43(t0, 2, 256, None, "mask3")
                    level2(t0)

            if WIDE:
                level43(0, 4, 512, None, "mask4")
                level43(0, 4, 256, None, "mask3")
                level2_wide()

            # D1 (a pairs: t=2 into t=0); no m update
            mask1 = pool_mask.tile([128, 256], U8, tag="mask1")
            for _ in range(REP_DVE):
                nc.vector.tensor_tensor(
                    mask1[:, :], mt[:, 2, 0:256], mt[:, 0, 0:256], mybir.AluOpType.is_gt
                )
            mk = mask1.unsqueeze(1).broadcast_to((128, 2, 256))
            for _ in range(REP_DVE):
                nc.vector.copy_predicated(xtr[:, 0, :, 0:256], mk, xtr[:, 2, :, 0:256])

            # store winners (slabs j=0 r, j=1 i; [0:256) each)
            if OUT_STAGE:
                OUTT = pool_mask.tile([128, 512], F32, tag="OUT")
                nc.scalar.activation(
                    OUTT.rearrange("p (ri f) -> p ri f", ri=2),
                    xtr[:, 0, :, 0:256], mybir.ActivationFunctionType.Copy,
                )
                nc.sync.dma_start(out=y_dram[k], in_=OUTT[:, :])
            else:
                nc.sync.dma_start(out=y_dram[k], in_=xtr[:, 0, :, 0:256])

    nc.compile()
    return nc


def _get_nc():
    global _COMPILED
    if _COMPILED is None:
        _COMPILED = _build()
    return _COMPILED


def _prep_core(xb: np.ndarray) -> np.ndarray:
    """xb: [2, 32, 16,16,16,16] -> [4, 128, 8192] slab-packed, parity-split."""
    # [ri, chunk, c8, o1, a, o2, b, o3, s, o4, d]
    t = xb.reshape(RI, C // NCH, NCH, O, 2, O, 2, O, 2, O, 2)
    # -> [chunk, a, b, ri, c8, o1, o2, d, s, o3, o4]
    t = t.transpose(1, 4, 6, 0, 2, 3, 5, 10, 8, 7, 9)
    # merge (o1,o2) -> split (hi, q)
    t = t.reshape(C // NCH, 2, 2, RI, NCH, 16, 4, 2, 2, O, O)
    # -> [chunk, c8, hi, a, b, ri, d, s, q, o3, o4]
    t = t.transpose(0, 4, 5, 1, 2, 3, 7, 8, 6, 9, 10)
    return np.ascontiguousarray(t).reshape(C // NCH, 128, XF)


def _post_core(y: np.ndarray) -> np.ndarray:
    """y: [4, 128, 512] -> [2, 32, 8, 8, 8, 8]."""
    # [chunk, c8, hi, ri, q, o3o4]
    yk = y.reshape(C // NCH, NCH, 16, RI, 4, O * O)
    out = yk.transpose(3, 0, 1, 2, 4, 5).reshape(RI, C, 16 * 4, O * O)
    return out.reshape(RI, C, O, O, O, O)


def _run(inputs_x: np.ndarray, trace: bool = False):
    nc = _get_nc()
    in_maps = [{"x": _prep_core(inputs_x[b])} for b in range(N_CORES)]
    last_err = None
    for _attempt in range(3):
        try:
            res = run_bass_kernel_spmd(nc, in_maps, list(range(N_CORES)), trace=trace)
            break
        except Exception as e:  # wedged-device retries
            last_err = e
            if "UNRECOVERABLE" not in str(e) and "UNAVAILABLE" not in str(e):
                raise
    else:
        raise last_err
    outs = np.empty((N_CORES, RI, C, O, O, O, O), dtype=np.float32)
    for b in range(N_CORES):
        outs[b] = _post_core(res.results[b]["y"])
    return outs, res


def kernel(input: np.ndarray) -> np.ndarray:
    input = np.asarray(input, dtype=np.float32)
    outs, _ = _run(input)
    return outs

